# revision 34
# baseline (speedup 1.0000x reference)
"""Multi-head causal attention (B=2, T=2048, C=1024, H=16, HS=64) on 8 TRN2
NeuronCores.

Sharding: 2 heads per core (tensor parallel). Each core receives the full
(pre-transposed) activations xT [B, C, T], its 2 heads' QKV weight slices
packed [C, 128], and its 128-column slice of w_proj transposed [128, C].
Each core computes a partial output [B, T, C] in bf16; the host sums the 8
partials and adds b_proj.

Per-core kernel (matmuls in float32r):
  - QT/KT [128(2 heads x 64), T] via lhsT=weight chunks, rhs=xT chunks.
  - V_aug [keys, 2, j, 64]: V (vo=0, via PE-transpose of VT) | ones (vo=1).
  - Flash-style causal attention in transposed layout: S^T[keys, q] blocks
    via lhsT=KT block, rhs=QT slice; exp on ScalarE (no max subtraction --
    scores are O(1) by construction); O^T = [V|1].T @ P^T accumulated over
    key blocks gives both O rows (0:64) and the softmax sums l (64:128).
  - Triangular masking of diagonal blocks via bf16 multiply on GpSimd.
  - Normalize with reciprocal_approx_fast; proj via lhsT=OhatT chunks.

Scheduling: the PE p-state ramps to 2.4 GHz only after ~3us of gap-free
execution and resets on any idle, so the emission interleaves one "filler"
PE unit (a QKV 512-col chain or a proj chunk) into every attention jg slot,
and skews O^T one jg behind S^T/exp so the PE never waits on ScalarE.
"""

import math
import sys
from collections import deque
from contextlib import ExitStack

if "/opt/trn_rl_repo" not in sys.path:
    sys.path.insert(0, "/opt/trn_rl_repo")

import numpy as np

import concourse.mybir as mybir
import concourse.tile as tile
from concourse import bacc
from concourse.bass import ts
from concourse.bass_utils import run_bass_kernel_spmd
from concourse.tile_rust import add_dep_helper

B, T, C = 2, 2048, 1024
H, HS = 16, 64
NCORES = 8
HPC = H // NCORES  # heads per core
P = 128
G = 512  # q-group size
NG = T // G
KB = 128  # key block
NKB = T // KB
NPO = C // P  # contraction chunks
F32 = mybir.dt.float32
F32R = mybir.dt.float32r
BF16 = mybir.dt.bfloat16
SCALE = float(HS) ** -0.5

_nc_cache = {}


def _emit(tc):
    nc = tc.nc
    xt = nc.dram_tensor("xt", [B, C, T], BF16, kind="ExternalInput").ap()
    wq2 = nc.dram_tensor("wq2", [C, 128], BF16, kind="ExternalInput").ap()
    wk2 = nc.dram_tensor("wk2", [C, 128], BF16, kind="ExternalInput").ap()
    wv2 = nc.dram_tensor("wv2", [C, 128], BF16, kind="ExternalInput").ap()
    wpt = nc.dram_tensor("wpt", [128, C], BF16, kind="ExternalInput").ap()
    trid = nc.dram_tensor("tri", [P, P], BF16, kind="ExternalInput").ap()
    identd = nc.dram_tensor("ident", [P, 64], BF16, kind="ExternalInput").ap()
    onesd = nc.dram_tensor("ones", [P, NKB, 64], BF16, kind="ExternalInput").ap()
    out = nc.dram_tensor("out", [B, T, C], BF16, kind="ExternalOutput").ap()

    ctx = ExitStack()
    persist = ctx.enter_context(tc.tile_pool(name="persist", bufs=1))
    xt_pool = ctx.enter_context(tc.tile_pool(name="xtp", bufs=2))
    qk_pool = ctx.enter_context(tc.tile_pool(name="qkp", bufs=2))
    vt_pool = ctx.enter_context(tc.tile_pool(name="vtp", bufs=2))
    vaug_pool = ctx.enter_context(tc.tile_pool(name="vaugp", bufs=2))
    pt_pool = ctx.enter_context(tc.tile_pool(name="ptp", bufs=4))
    norm_pool = ctx.enter_context(tc.tile_pool(name="normp", bufs=2))
    ohat_pool = ctx.enter_context(tc.tile_pool(name="ohatp", bufs=2))
    out_pool = ctx.enter_context(tc.tile_pool(name="outp", bufs=3))
    st_psum = ctx.enter_context(tc.tile_pool(name="stps", bufs=2, space="PSUM"))
    ot_psum = ctx.enter_context(tc.tile_pool(name="otps", bufs=2, space="PSUM"))
    mm_psum = ctx.enter_context(tc.tile_pool(name="mmps", bufs=2, space="PSUM"))

    wq_sb = persist.tile([P, NPO, 128], BF16, tag="wq")
    wk_sb = persist.tile([P, NPO, 128], BF16, tag="wk")
    wv_sb = persist.tile([P, NPO, 128], BF16, tag="wv")
    wpt_sb = persist.tile([P, C], BF16, tag="wpt")
    tri_sb = persist.tile([P, P], BF16, tag="tri")
    ident = persist.tile([P, 64], BF16, tag="ident")

    # ---- xt loading ----
    # pi-major layout: partition pi holds x^T rows 8*pi..8*pi+7, so each
    # quarter-DMA moves 4 rows x 2KB contiguous per partition (descriptor-
    # efficient); subtile deps let QKV chain (b, tg) start once the two
    # po-half DMAs covering its t-half have landed
    xt_tiles = {}
    xt_dmas = []

    def load_xt_piece(b, ph, t0, t1):
        t = xt_tiles[b]
        src = xt[b].rearrange("(pi po) t -> pi po t", po=NPO)
        i = nc.sync.dma_start(
            t[:, 4 * ph : 4 * ph + 4, t0:t1],
            src[:, 4 * ph : 4 * ph + 4, t0:t1],
        )
        if len(xt_dmas) >= 2:
            add_dep_helper(i.ins, xt_dmas[-2].ins, sync=True)
        xt_dmas.append(i)

    for b in (0, 1):
        xt_tiles[b] = xt_pool.tile([P, NPO, T], BF16, tag="xt", name=f"xt{b}")

    # order matches the merged g-interleaved schedule: b0 tg0 cols first
    # (unblocks the eager QKV chain), then alternate batches just-in-time
    nc.sync.dma_start(wq_sb[:], wq2.rearrange("(pi po) d -> pi po d", po=NPO))
    load_xt_piece(0, 0, 0, 512)
    load_xt_piece(0, 1, 0, 512)
    nc.sync.dma_start(wk_sb[:], wk2.rearrange("(pi po) d -> pi po d", po=NPO))
    nc.sync.dma_start(wv_sb[:], wv2.rearrange("(pi po) d -> pi po d", po=NPO))
    load_xt_piece(0, 0, 512, 1024)
    load_xt_piece(0, 1, 512, 1024)
    nc.sync.dma_start(ident[:], identd[:])
    nc.sync.dma_start(tri_sb[:], trid[:])
    load_xt_piece(1, 0, 0, 1024)
    load_xt_piece(1, 1, 0, 1024)
    nc.sync.dma_start(wpt_sb[:], wpt[:])
    load_xt_piece(0, 0, 1024, 2048)
    load_xt_piece(0, 1, 1024, 2048)
    load_xt_piece(1, 0, 1024, 2048)
    load_xt_piece(1, 1, 1024, 2048)

    def new_state(b):
        st = {
            "b": b,
            "qt": qk_pool.tile([P, T], BF16, tag="qt", name=f"qt{b}"),
            "kt": qk_pool.tile([P, T], BF16, tag="kt", name=f"kt{b}"),
            "vt": vt_pool.tile([P, T], BF16, tag="vt", name=f"vt{b}"),
            "ohat": ohat_pool.tile([P, T], BF16, tag="ohat", name=f"oh{b}"),
            "vaug": [],
        }
        for h in range(HPC):
            va = vaug_pool.tile(
                [P, NKB, 128], BF16, tag=f"vaug{h}", name=f"va{b}{h}"
            )
            # separate (gpsimd-engine) DMA queue; delayed so it doesn't
            # steal HBM bandwidth from the startup-critical xt tg0/tg1 loads
            i = nc.gpsimd.dma_start(va[:, :, 64:128], onesd[:])
            add_dep_helper(i.ins, xt_dmas[1 if b == 0 else 5].ins, sync=True)
            st["vaug"].append(va)
        return st

    # total exps = 2 heads x 2 batches x sum_g(2g+2) = 80
    phase = {"exps_left": 80, "flip": 0}

    # ---------- building blocks ----------
    def emit_qkv_group(st, which, tg, copy_eng):
        w_sb, dst = {
            "q": (wq_sb, st["qt"]),
            "k": (wk_sb, st["kt"]),
            "v": (wv_sb, st["vt"]),
        }[which]
        ps = mm_psum.tile([P, 512], F32, tag="mm", name=f"qkv{which}{tg}")
        xtt = xt_tiles[st["b"]]
        for po in range(NPO):
            nc.tensor.matmul(
                ps[:],
                w_sb[:, po, :],
                xtt[:, po, ts(tg, 512)],
                start=(po == 0),
                stop=(po == NPO - 1),
            )
        if copy_eng == "scalar":
            nc.scalar.copy(dst[:, ts(tg, 512)], ps[:])
        else:
            nc.vector.tensor_copy(dst[:, ts(tg, 512)], ps[:])

    def emit_vaug_part(st, tg):
        vaug = st["vaug"]
        tps = [
            mm_psum.tile([P, 4, 64], BF16, tag="mm", name=f"vtr{h}")
            for h in range(HPC)
        ]
        for kk in range(4):
            kb = 4 * tg + kk
            for h in range(HPC):
                nc.tensor.transpose(
                    tps[h][:, kk, :],
                    st["vt"][64 * h : 64 * h + 64, ts(kb, KB)],
                    ident[64 * h : 64 * h + 64, :],
                )
        for h in range(HPC):
            nc.vector.tensor_copy(
                vaug[h][:, 4 * tg : 4 * tg + 4, 0:64], tps[h][:]
            )

    def emit_proj_chunk(st, g, tc4, copy_eng):
        b, ohat = st["b"], st["ohat"]
        t0 = G * g + P * tc4
        o_sb = out_pool.tile([P, C], BF16, tag="osb", name=f"osb{b}{g}{tc4}")
        for n in range(C // 512):
            pj = mm_psum.tile([P, 512], F32, tag="mm", name=f"pj{n}")
            nc.tensor.matmul(
                pj[:],
                ohat[:, t0 : t0 + P],
                wpt_sb[:, ts(n, 512)],
                start=True,
                stop=True,
            )
            eng = copy_eng
            if eng == "auto":
                # ScalarE is saturated by exps until the attention tail
                if phase["exps_left"] > 8:
                    eng = "vector"
                else:
                    phase["flip"] ^= 1
                    eng = "scalar" if phase["flip"] else "vector"
            if eng == "scalar":
                nc.scalar.copy(o_sb[:, ts(n, 512)], pj[:])
            else:
                nc.vector.tensor_copy(o_sb[:, ts(n, 512)], pj[:])
        nc.sync.dma_start(out[b, t0 : t0 + P, :], o_sb[:])

    # ---------- filler unit queue ----------
    # each unit: (key, fn); key=(b, tg) for qkv units (forced before the
    # attention group that needs them, pop-eligible per-phase so the late
    # batch's QKV pads the scalar-bound second half), (-1, -1) for proj
    # units (always eligible once queued)
    units = []

    def pop_units(maxn, allow):
        n = 0
        i = 0
        while i < len(units) and n < maxn:
            if units[i][0] <= allow:
                _, fn = units.pop(i)
                fn()
                n += 1
            else:
                i += 1

    def force_units(b, g):
        i = 0
        while i < len(units):
            if units[i][0] <= (b, g):
                _, fn = units.pop(i)
                fn()
            else:
                i += 1

    def queue_qkv(st, tg):
        for which in ("q", "k", "v"):
            units.append(
                ((st["b"], tg),
                 lambda st=st, w=which, tg=tg: emit_qkv_group(st, w, tg, "vector"))
            )
        units.append(((st["b"], tg), lambda st=st, tg=tg: emit_vaug_part(st, tg)))

    def queue_proj(st, g, copy_eng="auto"):
        for tc4 in range(G // P):
            units.append(
                ((-1, -1),
                 lambda st=st, g=g, tc4=tc4, e=copy_eng: emit_proj_chunk(st, g, tc4, e))
            )

    # ---------- attention for one (b, g) with one-jg S/exp -> O skew ----------
    def emit_attn_g(st, g):
        b, qt, kt, vaug, ohat = st["b"], st["qt"], st["kt"], st["vaug"], st["ohat"]
        n_j = 4 * g + 4
        n_jg = n_j // 2
        otps_h = [
            ot_psum.tile([P, G], F32, tag="ot", name=f"ot{b}{g}{h}")
            for h in range(HPC)
        ]
        pend = None  # (js, pt_h) waiting for O^T

        def emit_s_exp(jg):
            js = (2 * jg, 2 * jg + 1)
            stps_h = [
                st_psum.tile([P, 2, G], F32, tag="st", name=f"st{b}{g}{h}")
                for h in range(HPC)
            ]
            pt_h = [
                pt_pool.tile([P, 2, G], BF16, tag=f"pt{h}", name=f"pt{b}{g}{h}")
                for h in range(HPC)
            ]
            for idx, j in enumerate(js):
                r = j - 4 * g
                q0 = 128 * r if r >= 0 else 0
                for h in range(HPC):
                    hb = 64 * h
                    nc.tensor.matmul(
                        stps_h[h][:, idx, q0:G],
                        kt[hb : hb + 64, ts(j, KB)],
                        qt[hb : hb + 64, G * g + q0 : G * (g + 1)],
                        start=True,
                        stop=True,
                    )
            # skip exp for columns no O^T matmul will read (above-diagonal
            # q < q0 of the first block in the pair)
            qmin = max(0, 128 * (js[0] - 4 * g))
            for h in range(HPC):
                nc.scalar.activation(
                    pt_h[h][:, :, qmin:G],
                    stps_h[h][:, :, qmin:G],
                    mybir.ActivationFunctionType.Exp,
                    scale=SCALE,
                )
            phase["exps_left"] -= HPC
            # causal mask on the diagonal boundary blocks (cheap on DVE; the
            # one-jg S/exp->O skew gives this slack)
            for idx, j in enumerate(js):
                r = j - 4 * g
                if r >= 0:
                    q0 = 128 * r
                    for h in range(HPC):
                        nc.vector.tensor_tensor(
                            pt_h[h][:, idx, q0 : q0 + 128],
                            pt_h[h][:, idx, q0 : q0 + 128],
                            tri_sb[:],
                            mybir.AluOpType.mult,
                        )
            return (js, pt_h)

        def emit_o(pend):
            js, pt_h = pend
            for idx, j in enumerate(js):
                r = j - 4 * g
                q0 = 128 * r if r >= 0 else 0
                for h in range(HPC):
                    nc.tensor.matmul(
                        otps_h[h][:, q0:G],
                        vaug[h][:, j, :],
                        pt_h[h][:, idx, q0:G],
                        start=(j == 0),
                        stop=(j == n_j - 1),
                    )

        allow = (1, 3)
        maxn = 2
        for jg in range(n_jg + 1):
            if jg < n_jg:
                pend_new = emit_s_exp(jg)
            if jg < n_jg:
                # skip the final slot: popping fillers there would queue their
                # vector copies ahead of this group's norm ops
                pop_units(maxn, allow)
            if pend is not None:
                emit_o(pend)
            pend = pend_new if jg < n_jg else None

        # normalize: O rows (0:64 per head) / l rows (64:128 per head)
        l_sb = norm_pool.tile([P, G], F32, tag="lsb", name=f"l{b}{g}")
        rinv = norm_pool.tile([P, G], F32, tag="rinv", name=f"r{b}{g}")
        stag = norm_pool.tile([P, G], F32, tag="stag", name=f"sg{b}{g}")
        for h in range(HPC):
            hb = 64 * h
            nc.vector.tensor_copy(stag[hb : hb + 64, :], otps_h[h][0:64, :])
            nc.vector.tensor_copy(l_sb[hb : hb + 64, :], otps_h[h][64:128, :])
        nc.vector.reciprocal_approx_fast(rinv[:], l_sb[:])
        nc.vector.tensor_tensor(
            ohat[:, ts(g, G)], stag[:], rinv[:], mybir.AluOpType.mult
        )
        pop_units(maxn, allow)

    # ================= emission =================
    st = {0: new_state(0), 1: new_state(1)}

    # eager: qkv b0 tg0 (copies split scalar/vector: both engines idle here)
    for i, which in enumerate(("q", "k", "v")):
        emit_qkv_group(st[0], which, 0, "scalar" if i % 2 == 0 else "vector")
    emit_vaug_part(st[0], 0)

    for b in (0,):
        for tg in (1, 2, 3):
            queue_qkv(st[b], tg)
    for tg in range(NG):
        queue_qkv(st[1], tg)

    # batches interleaved at the g level (b1 staggered one group later so
    # its xt DMA arrives in time): spreads exp (Scalar) and copy (Vector)
    # load uniformly so no phase saturates an aux engine
    for b, g in [(0, 0), (0, 1), (1, 0), (1, 1), (0, 2), (1, 2), (0, 3), (1, 3)]:
        force_units(b, g)
        emit_attn_g(st[b], g)
        queue_proj(st[b], g)

    # tail drain: alternate copy engines (no exps left, scalar is free)
    while units:
        _, fn = units.pop(0)
        fn()
    ctx.close()


def _build():
    if "nc" in _nc_cache:
        return _nc_cache["nc"]
    nc = bacc.Bacc("TRN2", target_bir_lowering=False, debug=False)
    with tile.TileContext(nc) as tc:
        _emit(tc)
    nc.compile()
    _nc_cache["nc"] = nc
    return nc


def _make_in_maps(x, wq, wk, wv, w_proj):
    import ml_dtypes

    bf16 = ml_dtypes.bfloat16
    xt = np.ascontiguousarray(x.transpose(0, 2, 1)).astype(bf16)
    tri = np.triu(np.ones((P, P), dtype=np.float32)).astype(bf16)
    ident = np.tile(np.eye(64, dtype=np.float32), (2, 1)).astype(bf16)
    ones = np.ones((P, NKB, 64), dtype=np.float32).astype(bf16)
    in_maps = []
    for c in range(NCORES):
        h0 = HPC * c
        in_maps.append(
            {
                "xt": xt,
                "wq2": np.ascontiguousarray(
                    np.concatenate([wq[h0 + i] for i in range(HPC)], axis=1)
                ).astype(bf16),
                "wk2": np.ascontiguousarray(
                    np.concatenate([wk[h0 + i] for i in range(HPC)], axis=1)
                ).astype(bf16),
                "wv2": np.ascontiguousarray(
                    np.concatenate([wv[h0 + i] for i in range(HPC)], axis=1)
                ).astype(bf16),
                "wpt": np.ascontiguousarray(
                    w_proj[:, 128 * c : 128 * (c + 1)].T
                ).astype(bf16),
                "tri": tri,
                "ident": ident,
                "ones": ones,
            }
        )
    return in_maps


def kernel(x, wq, wk, wv, w_proj, b_proj):
    x = np.asarray(x, dtype=np.float32)
    wq = np.asarray(wq, dtype=np.float32)
    wk = np.asarray(wk, dtype=np.float32)
    wv = np.asarray(wv, dtype=np.float32)
    w_proj = np.asarray(w_proj, dtype=np.float32)
    b_proj = np.asarray(b_proj, dtype=np.float32)

    nc = _build()
    in_maps = _make_in_maps(x, wq, wk, wv, w_proj)
    res = run_bass_kernel_spmd(nc, in_maps, core_ids=list(range(NCORES)))
    acc = np.zeros((B, T, C), dtype=np.float64)
    for r in res.results:
        acc += np.asarray(r["out"], dtype=np.float64)
    return (acc + b_proj).astype(np.float32)


# revision 35
# speedup vs baseline: 1.0485x; 1.0485x over previous
"""Multi-head causal attention (B=2, T=2048, C=1024, H=16, HS=64) on 8 TRN2
NeuronCores.

Sharding: (batch, head-group) grid — core c handles batch c//4 and heads
4*(c%4)..4*(c%4)+3. Each core receives its batch's pre-transposed
activations xT [C, T] in bf16, its 4 heads' QKV weight slices packed
[C, 256], and its 256-row slice of w_proj^T [256, C]. Each core computes a
partial output [T, C] in bf16; the host sums 4 partials per batch and adds
b_proj. Versus head-only sharding this halves per-core xt DMA, proj copy
work, and output DMA.

Per-core kernel (all matmuls bf16 — f32r streams at ~2 cycles/row on TRN2
silicon, bf16 at 1):
  - The 4 heads are processed as two head-pairs p=0,1, giving two
    independent attention pipelines (PSUM fits exactly two).
  - QT/KT [128(2 heads x 64), T] per pair via lhsT=weight chunks, rhs=xT.
  - V_aug [keys, j, V|1]: V via PE-transpose of VT, ones via DMA.
  - Flash-style causal attention in transposed layout: S^T[keys, q] blocks
    via lhsT=KT block, rhs=QT slice; exp on ScalarE (no max subtraction --
    scores are O(1) by construction); O^T = [V|1].T @ P^T accumulated over
    key blocks gives both O rows (0:64) and the softmax sums l (64:128).
  - Causal masking of diagonal blocks via bf16 tri multiply on DVE.
  - Normalize with reciprocal_approx_fast; proj contracts both pairs'
    ohat (2-matmul chains) against wpt halves.

Scheduling: the PE HAM clock gate reaches 2.4 GHz only under sustained
activity (3.4us windows) and halves the clock after idle ones, so the
emission (a) interleaves the two head-pairs' attention groups, (b) pops
"filler" PE units (QKV 512-col chains / proj chunks) into every attention
jg slot, and (c) skews O^T one jg behind S^T/exp so the PE never waits on
ScalarE.
"""

import sys
from contextlib import ExitStack

if "/opt/trn_rl_repo" not in sys.path:
    sys.path.insert(0, "/opt/trn_rl_repo")

import numpy as np

import concourse.mybir as mybir
import concourse.tile as tile
from concourse import bacc
from concourse.bass import ts
from concourse.bass_utils import run_bass_kernel_spmd
from concourse.tile_rust import add_dep_helper

B, T, C = 2, 2048, 1024
H, HS = 16, 64
NCORES = 8
HPC = 4  # heads per core
P = 128
G = 512  # q-group size
NG = T // G
KB = 128  # key block
NKB = T // KB
NPO = C // P  # contraction chunks
F32 = mybir.dt.float32
BF16 = mybir.dt.bfloat16
SCALE = float(HS) ** -0.5

_nc_cache = {}


def _emit(tc):
    nc = tc.nc
    xt = nc.dram_tensor("xt", [C, T], BF16, kind="ExternalInput").ap()
    wq2 = nc.dram_tensor("wq2", [C, 256], BF16, kind="ExternalInput").ap()
    wk2 = nc.dram_tensor("wk2", [C, 256], BF16, kind="ExternalInput").ap()
    wv2 = nc.dram_tensor("wv2", [C, 256], BF16, kind="ExternalInput").ap()
    wpt = nc.dram_tensor("wpt", [256, C], BF16, kind="ExternalInput").ap()
    trid = nc.dram_tensor("tri", [P, P], BF16, kind="ExternalInput").ap()
    identd = nc.dram_tensor("ident", [P, 64], BF16, kind="ExternalInput").ap()
    onesd = nc.dram_tensor("ones", [P, NKB, 64], BF16, kind="ExternalInput").ap()
    out = nc.dram_tensor("out", [T, C], BF16, kind="ExternalOutput").ap()

    ctx = ExitStack()
    persist = ctx.enter_context(tc.tile_pool(name="persist", bufs=1))
    xt_pool = ctx.enter_context(tc.tile_pool(name="xtp", bufs=1))
    qk_pool = ctx.enter_context(tc.tile_pool(name="qkp", bufs=2))
    vt_pool = ctx.enter_context(tc.tile_pool(name="vtp", bufs=2))
    vaug_pool = ctx.enter_context(tc.tile_pool(name="vaugp", bufs=2))
    pt_pool = ctx.enter_context(tc.tile_pool(name="ptp", bufs=4))
    norm_pool = ctx.enter_context(tc.tile_pool(name="normp", bufs=2))
    ohat_pool = ctx.enter_context(tc.tile_pool(name="ohatp", bufs=2))
    out_pool = ctx.enter_context(tc.tile_pool(name="outp", bufs=3))
    st_psum = ctx.enter_context(tc.tile_pool(name="stps", bufs=2, space="PSUM"))
    ot_psum = ctx.enter_context(tc.tile_pool(name="otps", bufs=2, space="PSUM"))
    mm_psum = ctx.enter_context(tc.tile_pool(name="mmps", bufs=2, space="PSUM"))

    wq_sb = persist.tile([P, NPO, 256], BF16, tag="wq")
    wk_sb = persist.tile([P, NPO, 256], BF16, tag="wk")
    wv_sb = persist.tile([P, NPO, 256], BF16, tag="wv")
    wpt_sb = persist.tile([P, 2, C], BF16, tag="wpt")
    tri_sb = persist.tile([P, P], BF16, tag="tri")
    ident = persist.tile([P, 64], BF16, tag="ident")

    # ---- xt loading ----
    # pi-major layout: partition pi holds x^T rows 8*pi..8*pi+7 so pieces
    # move 1-2KB contiguous per partition per row (descriptor-efficient);
    # subtile deps let QKV chain tg start once its pieces have landed
    xtt = xt_pool.tile([P, NPO, T], BF16, tag="xt", name="xtt")
    xt_dmas = []

    def load_xt_piece(ph, t0, t1):
        src = xt.rearrange("(pi po) t -> pi po t", po=NPO)
        i = nc.sync.dma_start(
            xtt[:, 4 * ph : 4 * ph + 4, t0:t1],
            src[:, 4 * ph : 4 * ph + 4, t0:t1],
        )
        if len(xt_dmas) >= 2:
            add_dep_helper(i.ins, xt_dmas[-2].ins, sync=True)
        xt_dmas.append(i)

    nc.sync.dma_start(wq_sb[:], wq2.rearrange("(pi po) d -> pi po d", po=NPO))
    load_xt_piece(0, 0, 512)
    load_xt_piece(1, 0, 512)
    nc.sync.dma_start(wk_sb[:], wk2.rearrange("(pi po) d -> pi po d", po=NPO))
    nc.sync.dma_start(wv_sb[:], wv2.rearrange("(pi po) d -> pi po d", po=NPO))
    load_xt_piece(0, 512, 1024)
    load_xt_piece(1, 512, 1024)
    nc.sync.dma_start(ident[:], identd[:])
    nc.sync.dma_start(tri_sb[:], trid[:])
    nc.sync.dma_start(wpt_sb[:], wpt.rearrange("(w pi) c -> pi w c", pi=P))
    load_xt_piece(0, 1024, 2048)
    load_xt_piece(1, 1024, 2048)

    def new_state(p):
        st = {
            "p": p,
            "qt": qk_pool.tile([P, T], BF16, tag="qt", name=f"qt{p}"),
            "kt": qk_pool.tile([P, T], BF16, tag="kt", name=f"kt{p}"),
            "vt": vt_pool.tile([P, T], BF16, tag="vt", name=f"vt{p}"),
            "ohat": ohat_pool.tile([P, T], BF16, tag="ohat", name=f"oh{p}"),
            "vaug": [],
        }
        for h in range(2):
            va = vaug_pool.tile(
                [P, NKB, 128], BF16, tag=f"vaug{h}", name=f"va{p}{h}"
            )
            # separate (gpsimd) DMA queue, delayed past the startup-critical
            # xt pieces
            i = nc.gpsimd.dma_start(va[:, :, 64:128], onesd[:])
            add_dep_helper(i.ins, xt_dmas[1].ins, sync=True)
            st["vaug"].append(va)
        return st

    # total exps = 2 heads x 2 pairs x sum_g(2g+2) = 80
    phase = {"exps_left": 80, "flip": 0}

    # ---------- building blocks ----------
    def emit_qkv_group(st, which, tg, copy_eng):
        w_sb, dst = {
            "q": (wq_sb, st["qt"]),
            "k": (wk_sb, st["kt"]),
            "v": (wv_sb, st["vt"]),
        }[which]
        p = st["p"]
        ps = mm_psum.tile([P, 512], F32, tag="mm", name=f"qkv{which}{tg}")
        for po in range(NPO):
            nc.tensor.matmul(
                ps[:],
                w_sb[:, po, 128 * p : 128 * p + 128],
                xtt[:, po, ts(tg, 512)],
                start=(po == 0),
                stop=(po == NPO - 1),
            )
        if copy_eng == "scalar":
            nc.scalar.copy(dst[:, ts(tg, 512)], ps[:])
        else:
            nc.vector.tensor_copy(dst[:, ts(tg, 512)], ps[:])

    def emit_vaug_part(st, tg):
        vaug = st["vaug"]
        tps = [
            mm_psum.tile([P, 4, 64], BF16, tag="mm", name=f"vtr{h}")
            for h in range(2)
        ]
        for kk in range(4):
            kb = 4 * tg + kk
            for h in range(2):
                nc.tensor.transpose(
                    tps[h][:, kk, :],
                    st["vt"][64 * h : 64 * h + 64, ts(kb, KB)],
                    ident[64 * h : 64 * h + 64, :],
                )
        for h in range(2):
            nc.vector.tensor_copy(
                vaug[h][:, 4 * tg : 4 * tg + 4, 0:64], tps[h][:]
            )

    def emit_proj_chunk(sts, g, tc4, copy_eng):
        t0 = G * g + P * tc4
        o_sb = out_pool.tile([P, C], BF16, tag="osb", name=f"osb{g}{tc4}")
        for n in range(C // 512):
            pj = mm_psum.tile([P, 512], F32, tag="mm", name=f"pj{n}")
            nc.tensor.matmul(
                pj[:],
                sts[0]["ohat"][:, t0 : t0 + P],
                wpt_sb[:, 0, ts(n, 512)],
                start=True,
                stop=False,
            )
            nc.tensor.matmul(
                pj[:],
                sts[1]["ohat"][:, t0 : t0 + P],
                wpt_sb[:, 1, ts(n, 512)],
                start=False,
                stop=True,
            )
            eng = copy_eng
            if eng == "auto":
                # ScalarE is saturated by exps until the attention tail
                if phase["exps_left"] > 8:
                    eng = "vector"
                else:
                    phase["flip"] ^= 1
                    eng = "scalar" if phase["flip"] else "vector"
            if eng == "scalar":
                nc.scalar.copy(o_sb[:, ts(n, 512)], pj[:])
            else:
                nc.vector.tensor_copy(o_sb[:, ts(n, 512)], pj[:])
        nc.sync.dma_start(out[t0 : t0 + P, :], o_sb[:])

    # ---------- filler unit queue ----------
    # each unit: (key, fn); key=(p, tg) for qkv units (forced before the
    # attention group that needs them), (-1, -1) for proj units
    units = []

    def pop_units(maxn, allow=(9, 9)):
        n = 0
        i = 0
        while i < len(units) and n < maxn:
            if units[i][0] <= allow:
                _, fn = units.pop(i)
                fn()
                n += 1
            else:
                i += 1

    def force_units(p, g):
        i = 0
        while i < len(units):
            if units[i][0] <= (p, g):
                _, fn = units.pop(i)
                fn()
            else:
                i += 1

    def queue_qkv(st, tg):
        for which in ("q", "k", "v"):
            units.append(
                ((st["p"], tg),
                 lambda st=st, w=which, tg=tg: emit_qkv_group(st, w, tg, "vector"))
            )
        units.append(((st["p"], tg), lambda st=st, tg=tg: emit_vaug_part(st, tg)))

    def queue_proj(sts, g, copy_eng="auto"):
        for tc4 in range(G // P):
            units.append(
                ((-1, -1),
                 lambda g=g, tc4=tc4, e=copy_eng: emit_proj_chunk(sts, g, tc4, e))
            )

    # ---------- attention for one (pair, g) with one-jg S/exp -> O skew ----
    def emit_attn_g(st, g):
        p, qt, kt, vaug, ohat = st["p"], st["qt"], st["kt"], st["vaug"], st["ohat"]
        n_j = 4 * g + 4
        n_jg = n_j // 2
        otps_h = [
            ot_psum.tile([P, G], F32, tag="ot", name=f"ot{p}{g}{h}")
            for h in range(2)
        ]
        pend = None  # (js, pt_h) waiting for O^T

        def emit_s_exp(jg):
            js = (2 * jg, 2 * jg + 1)
            stps_h = [
                st_psum.tile([P, 2, G], F32, tag="st", name=f"st{p}{g}{h}")
                for h in range(2)
            ]
            pt_h = [
                pt_pool.tile([P, 2, G], BF16, tag=f"pt{h}", name=f"pt{p}{g}{h}")
                for h in range(2)
            ]
            for idx, j in enumerate(js):
                r = j - 4 * g
                q0 = 128 * r if r >= 0 else 0
                for h in range(2):
                    hb = 64 * h
                    nc.tensor.matmul(
                        stps_h[h][:, idx, q0:G],
                        kt[hb : hb + 64, ts(j, KB)],
                        qt[hb : hb + 64, G * g + q0 : G * (g + 1)],
                        start=True,
                        stop=True,
                    )
            # skip exp for columns no O^T matmul will read (above-diagonal
            # q < q0 of the first block in the pair)
            qmin = max(0, 128 * (js[0] - 4 * g))
            for h in range(2):
                nc.scalar.activation(
                    pt_h[h][:, :, qmin:G],
                    stps_h[h][:, :, qmin:G],
                    mybir.ActivationFunctionType.Exp,
                    scale=SCALE,
                )
            phase["exps_left"] -= 2
            # causal mask on the diagonal boundary blocks (cheap on DVE; the
            # one-jg S/exp->O skew gives this slack)
            for idx, j in enumerate(js):
                r = j - 4 * g
                if r >= 0:
                    q0 = 128 * r
                    for h in range(2):
                        nc.vector.tensor_tensor(
                            pt_h[h][:, idx, q0 : q0 + 128],
                            pt_h[h][:, idx, q0 : q0 + 128],
                            tri_sb[:],
                            mybir.AluOpType.mult,
                        )
            return (js, pt_h)

        def emit_o(pend):
            js, pt_h = pend
            for idx, j in enumerate(js):
                r = j - 4 * g
                q0 = 128 * r if r >= 0 else 0
                for h in range(2):
                    nc.tensor.matmul(
                        otps_h[h][:, q0:G],
                        vaug[h][:, j, :],
                        pt_h[h][:, idx, q0:G],
                        start=(j == 0),
                        stop=(j == n_j - 1),
                    )

        for jg in range(n_jg + 1):
            if jg < n_jg:
                pend_new = emit_s_exp(jg)
                # popping in the final slot would queue filler copies ahead
                # of this group's norm ops on the vector queue
                pop_units(2)
            if pend is not None:
                emit_o(pend)
            pend = pend_new if jg < n_jg else None

        # normalize: O rows (0:64 per head) / l rows (64:128 per head)
        l_sb = norm_pool.tile([P, G], F32, tag="lsb", name=f"l{p}{g}")
        rinv = norm_pool.tile([P, G], F32, tag="rinv", name=f"r{p}{g}")
        stag = norm_pool.tile([P, G], F32, tag="stag", name=f"sg{p}{g}")
        for h in range(2):
            hb = 64 * h
            nc.vector.tensor_copy(stag[hb : hb + 64, :], otps_h[h][0:64, :])
            nc.vector.tensor_copy(l_sb[hb : hb + 64, :], otps_h[h][64:128, :])
        nc.vector.reciprocal_approx_fast(rinv[:], l_sb[:])
        nc.vector.tensor_tensor(
            ohat[:, ts(g, G)], stag[:], rinv[:], mybir.AluOpType.mult
        )
        pop_units(2)

    # ================= emission =================
    sts = {0: new_state(0), 1: new_state(1)}

    # eager: qkv pair0 tg0 (copies split scalar/vector: both engines idle)
    for i, which in enumerate(("q", "k", "v")):
        emit_qkv_group(sts[0], which, 0, "scalar" if i % 2 == 0 else "vector")
    emit_vaug_part(sts[0], 0)

    for tg in (1, 2, 3):
        queue_qkv(sts[0], tg)
    for tg in range(NG):
        queue_qkv(sts[1], tg)

    # head-pairs interleaved at the g level: spreads exp (Scalar) and copy
    # (Vector) load uniformly; proj for group g queues once both pairs'
    # ohat g is normalized
    for p, g in [(0, 0), (1, 0), (0, 1), (1, 1), (0, 2), (1, 2), (0, 3), (1, 3)]:
        force_units(p, g)
        emit_attn_g(sts[p], g)
        if p == 1:
            queue_proj(sts, g)

    # tail drain
    while units:
        _, fn = units.pop(0)
        fn()
    ctx.close()


def _build():
    if "nc" in _nc_cache:
        return _nc_cache["nc"]
    nc = bacc.Bacc("TRN2", target_bir_lowering=False, debug=False)
    with tile.TileContext(nc) as tc:
        _emit(tc)
    nc.compile()
    _nc_cache["nc"] = nc
    return nc


def _make_in_maps(x, wq, wk, wv, w_proj):
    import ml_dtypes

    bf16 = ml_dtypes.bfloat16
    xtb = [np.ascontiguousarray(x[b].T).astype(bf16) for b in range(B)]
    tri = np.triu(np.ones((P, P), dtype=np.float32)).astype(bf16)
    ident = np.tile(np.eye(64, dtype=np.float32), (2, 1)).astype(bf16)
    ones = np.ones((P, NKB, 64), dtype=np.float32).astype(bf16)
    in_maps = []
    for c in range(NCORES):
        b, hg = c // 4, c % 4
        h0 = HPC * hg
        in_maps.append(
            {
                "xt": xtb[b],
                "wq2": np.ascontiguousarray(
                    np.concatenate([wq[h0 + i] for i in range(HPC)], axis=1)
                ).astype(bf16),
                "wk2": np.ascontiguousarray(
                    np.concatenate([wk[h0 + i] for i in range(HPC)], axis=1)
                ).astype(bf16),
                "wv2": np.ascontiguousarray(
                    np.concatenate([wv[h0 + i] for i in range(HPC)], axis=1)
                ).astype(bf16),
                "wpt": np.ascontiguousarray(
                    w_proj[:, 256 * hg : 256 * (hg + 1)].T
                ).astype(bf16),
                "tri": tri,
                "ident": ident,
                "ones": ones,
            }
        )
    return in_maps


def kernel(x, wq, wk, wv, w_proj, b_proj):
    x = np.asarray(x, dtype=np.float32)
    wq = np.asarray(wq, dtype=np.float32)
    wk = np.asarray(wk, dtype=np.float32)
    wv = np.asarray(wv, dtype=np.float32)
    w_proj = np.asarray(w_proj, dtype=np.float32)
    b_proj = np.asarray(b_proj, dtype=np.float32)

    nc = _build()
    in_maps = _make_in_maps(x, wq, wk, wv, w_proj)
    res = run_bass_kernel_spmd(nc, in_maps, core_ids=list(range(NCORES)))
    acc = np.zeros((B, T, C), dtype=np.float64)
    for c, r in enumerate(res.results):
        acc[c // 4] += np.asarray(r["out"], dtype=np.float64)
    return (acc + b_proj).astype(np.float32)
